# revision 3
# baseline (speedup 1.0000x reference)
"""Trainium2 Bass kernel for GCN+RNN (nn_GCNN_RNN_32461362823865).

Strategy (v2):
  - Host: dense normalized adjacency A^T (fp16, 3072-padded) from the edge
    list (exact reference remap semantics), fold W2 = W @ W_ih.T and
    c0 = b @ W_ih.T + b_ih + b_hh, pre-transpose/cast x_in per core.
  - Startup: x(s0..s2) DMA first (scalar queue), then A^T blocks (sync
    queue) interleaved with remaining x, so the PE chews z = x@W2 and U
    M-block 0 while the 31.5 MB input stream lands (DMA-bound ~95us).
  - U-phase: U^T = z^T x A^T in 7 M-blocks x (6+2 dc passes) x 24 kb,
    fp16 384-wide MMs at the 162ns N-roofline. Staging casts psum->fp16
    into 6 AllToAll rounds of (2,3,5,2,3,1) samples, each triggered as
    soon as its rows are staged, so U arrives node-sharded early.
  - RNN rides a ring buffer [128 part, 12 slots x 384]: rows 0:50 = h
    (written directly by ScalarE tanh), rows 64:114 = U (prestaged by
    GpSimd copies ahead of use; emission gated behind each round's
    collective trigger to keep the gpsimd queue deadlock-free). One
    128-contraction MM per step computes W_hh^T h + I*U; woven steps run
    1 MM + 1 ACT (N=384); tail steps split into column halves
    (2 MM + 2 ACT) to decouple latency. Output groups DMA straight out
    of the ring.

  Sample->core map: core c, round r holds global samples
  boff[r] + SR[r]*c + s4, boff = [0,16,40,80,96,120], SR = (2,3,5,2,3,1).
"""
import numpy as np

import concourse.bacc as bacc
import concourse.mybir as mybir
from concourse import tile
from concourse.bass_utils import run_bass_kernel_spmd

# ---- problem constants (hardcoded per contract) ----
N = 3070          # nodes
NP = 3072         # padded nodes (24 * 128, 8 * 384)
F = 128           # input features
J = 50            # folded feature dim (= RNN hidden)
B = 128           # batch (RNN sequence length)
NCORES = 8
S = B // NCORES   # samples per core = 16
NPC = NP // NCORES  # nodes per core = 384
KB = NP // 128    # 24 contraction blocks
SJ = S * J        # 800 U^T rows per core
NMB = 7           # M-blocks of 128 rows (last = 32)

SR = (2, 3, 5, 2, 3, 1)          # samples per round per core
R = len(SR)
BOFF = [0, 16, 40, 80, 96, 120]  # global step offset per round
ROFF = [0, 100, 250, 500, 600, 750, 800]   # U^T row offset per round
RJ = [SR[r] * J for r in range(R)]
TRIG_BLK = {1: 1, 2: 3, 3: 4, 4: 5, 5: 6}  # round -> M-block trigger (r0 in startup)
DCP = [list(range(6)), [6, 7]]   # dest-core passes (6+2 psum banks)

RING = 12         # rnn ring slots
TAIL_B = 112      # steps >= TAIL_B use the split (2 MM + 2 ACT) form
PRE = 8           # U prestage distance (steps ahead)

# weave schedule: M-block -> rnn steps woven into it (first 12 into pass1
# at 1-per-2kb, rest into pass2 at 1-per-kb)
WEAVE = {2: list(range(0, 16)), 3: list(range(16, 40)),
         5: list(range(40, 80)), 6: list(range(80, 112))}

F16 = mybir.dt.float16
F32 = mybir.dt.float32
TANH = mybir.ActivationFunctionType.Tanh

_PROGRAM_CACHE = {}


def _step_rc(b):
    """global step -> (round, src core, sample-within-block)."""
    r = 0
    while r + 1 < R and b >= BOFF[r + 1]:
        r += 1
    q = b - BOFF[r]
    return r, q // SR[r], q % SR[r]


# writeback groups aligned so ring slots never wrap mid-group:
# out[b] lives in slot (b+1) % RING rows 0:50.
WB_GROUPS = [(0, 3)] + [(3 + 4 * i, 4) for i in range(31)] + [(127, 1)]
WB_AFTER = {b0 + g - 1: (b0, g) for b0, g in WB_GROUPS}


def _build_program():
    if "nc" in _PROGRAM_CACHE:
        return _PROGRAM_CACHE["nc"]
    nc = bacc.Bacc("TRN2", target_bir_lowering=False, debug=False,
                   num_devices=NCORES)

    xT = nc.dram_tensor("xT", [S, F, N], F16, kind="ExternalInput")
    at = nc.dram_tensor("at", [NP, NP], F16, kind="ExternalInput")
    w2 = nc.dram_tensor("w2", [F, J], F16, kind="ExternalInput")
    ws = nc.dram_tensor("ws", [128, J], F16, kind="ExternalInput")
    c0 = nc.dram_tensor("c0", [J, 1], F32, kind="ExternalInput")
    h0T = nc.dram_tensor("h0T", [J, NPC], F16, kind="ExternalInput")
    out = nc.dram_tensor("out", [B, J, NPC], F16, kind="ExternalOutput")

    with tile.TileContext(nc) as tc:
        with (
            tc.tile_pool(name="consts", bufs=1) as consts,
            tc.tile_pool(name="persist", bufs=1) as persist,
            tc.tile_pool(name="dram", bufs=1, space="DRAM") as dram,
        ):
            w2_sb = consts.tile([F, J], F16, tag="w2_sb")
            ws_sb = consts.tile([128, J], F16, tag="ws_sb")
            c0_sb = consts.tile([J, 1], F32, tag="c0_sb")
            nc.sync.dma_start(w2_sb[:], w2[:])
            nc.sync.dma_start(ws_sb[:], ws[:])
            nc.sync.dma_start(c0_sb[:], c0[:])

            at_sb = persist.tile([128, KB * NP], F16, tag="at_sb")
            z_sb = persist.tile([128, KB * SJ], F16, tag="z_sb")

            a2a_in = [dram.tile([NCORES * RJ[r], NPC], F16, name=f"a2ai_{r}")
                      for r in range(R)]
            a2a_out = [dram.tile([NCORES * RJ[r], NPC], F16, name=f"a2ao_{r}")
                       for r in range(R)]

            upsum_ctx = tc.tile_pool(name="upsum", bufs=6, space="PSUM")
            upsum = upsum_ctx.__enter__()

            # ---- rnn state shared across phases ----
            state = {"ring": None, "pp_pool": None}
            u_tiles = {}
            trig_done = [False] * R
            pending_pre = []
            RC_ORDER = [(r, c) for r in range(R) for c in range(NCORES)]

            def load_ubig(r, c):
                u = state["upool"].tile([J, 5 * NPC], F16, tag="u",
                                        name=f"ubig_{r}_{c}")
                u_tiles[(r, c)] = u
                nc.gpsimd.dma_start(
                    u[:, 0:SR[r] * NPC].rearrange("j (s n) -> j s n",
                                                  s=SR[r]),
                    a2a_out[r][c * RJ[r]:(c + 1) * RJ[r], :].rearrange(
                        "(s j) n -> j s n", j=J))

            def _do_prestage(b):
                r, c, s4 = _step_rc(b)
                if (r, c) not in u_tiles:
                    load_ubig(r, c)
                if s4 == 0:
                    ix = RC_ORDER.index((r, c))
                    for nr, nc_ in RC_ORDER[ix + 1:ix + 2]:
                        if trig_done[nr] and (nr, nc_) not in u_tiles:
                            load_ubig(nr, nc_)
                slot = b % RING
                ring = state["ring"]
                nc.gpsimd.tensor_copy(
                    ring[64:64 + J, slot * NPC:(slot + 1) * NPC],
                    u_tiles[(r, c)][:, s4 * NPC:(s4 + 1) * NPC])

            def prestage_u(b):
                r = _step_rc(b)[0]
                if trig_done[r]:
                    _do_prestage(b)
                else:
                    pending_pre.append(b)

            def trig(r):
                nc.gpsimd.collective_compute(
                    "AllToAll", mybir.AluOpType.bypass,
                    replica_groups=[list(range(NCORES))],
                    ins=[a2a_in[r].opt()],
                    outs=[a2a_out[r].opt()])
                trig_done[r] = True
                ready = [b for b in pending_pre if trig_done[_step_rc(b)[0]]]
                for b in ready:
                    pending_pre.remove(b)
                    _do_prestage(b)

            def rnn_step(b):
                slot = b % RING
                nslot = (b + 1) % RING
                if b + PRE < B:
                    prestage_u(b + PRE)
                ring = state["ring"]
                rhs = ring[:, slot * NPC:(slot + 1) * NPC]
                dst = ring[0:J, nslot * NPC:(nslot + 1) * NPC]
                pp = state["pp_pool"].tile([J, NPC], F32, tag="pp",
                                           name=f"pp_{b}")
                if b < TAIL_B:
                    nc.tensor.matmul(pp[:], ws_sb[:], rhs,
                                     start=True, stop=True)
                    nc.scalar.activation(dst, pp[:], TANH,
                                         bias=c0_sb[:, 0:1])
                else:
                    H = NPC // 2
                    for half in range(2):
                        nc.tensor.matmul(
                            pp[:, half * H:(half + 1) * H], ws_sb[:],
                            rhs[:, half * H:(half + 1) * H],
                            start=True, stop=True)
                    for half in range(2):
                        nc.scalar.activation(
                            dst[:, half * H:(half + 1) * H],
                            pp[:, half * H:(half + 1) * H], TANH,
                            bias=c0_sb[:, 0:1])
                if b in WB_AFTER:
                    b0, g = WB_AFTER[b]
                    s0 = (b0 + 1) % RING
                    nc.sync.dma_start(
                        out[b0:b0 + g].rearrange("g j n -> j g n"),
                        ring[0:J, s0 * NPC:(s0 + g) * NPC].rearrange(
                            "j (g n) -> j g n", g=g))

            def stage(k, dc, st):
                """staging DMAs for M-block k, dest core dc (rows split
                at round boundaries)."""
                row0 = k * 128
                mrows = min(128, SJ - row0)
                for r in range(R):
                    lo = max(row0, ROFF[r])
                    hi = min(row0 + mrows, ROFF[r + 1])
                    if lo >= hi:
                        continue
                    nc.sync.dma_start(
                        a2a_in[r][dc * RJ[r] + lo - ROFF[r]:
                                  dc * RJ[r] + hi - ROFF[r], :],
                        st[lo - row0:hi - row0, :])

            # ================= startup: x/A DMA + z + U blk0 =============
            with nc.named_scope("startup"):
                with (
                    tc.tile_pool(name="xin", bufs=3) as xin,
                    tc.tile_pool(name="zpsum", bufs=2, space="PSUM") as zpsum,
                    tc.tile_pool(name="stg0", bufs=2) as stg0,
                ):
                    xbigs = {}

                    def load_x(s):
                        xb = xin.tile([F, NP], F16, tag="xbig",
                                      name=f"xbig_{s}")
                        xbigs[s] = xb
                        nc.scalar.dma_start(xb[:, 0:N], xT[s])
                        nc.vector.memset(xb[:, N:NP], 0.0)

                    def z_mm(s, kb):
                        zp = zpsum.tile([128, J], F32, tag="zp",
                                        name=f"zp_{s}_{kb}")
                        nc.tensor.matmul(
                            zp[:], xbigs[s][:, kb * 128:(kb + 1) * 128],
                            w2_sb[:], start=True, stop=True)
                        nc.vector.tensor_copy(
                            z_sb[:, kb * SJ + s * J:kb * SJ + (s + 1) * J],
                            zp[:])

                    # DMA order: x s0-2 (scalar q), A blocks (sync q)
                    # interleaved 2:1 with x s3-15
                    for s in range(3):
                        load_x(s)
                    xq = list(range(3, S))
                    for kb in range(KB):
                        nc.sync.dma_start(
                            at_sb[:, kb * NP:(kb + 1) * NP],
                            at[kb * 128:(kb + 1) * 128, :])
                        if kb % 2 == 0 and xq:
                            load_x(xq.pop(0))
                    while xq:
                        load_x(xq.pop(0))

                    # PE: z s0-2, then U blk0 pass1 kb-paced + z filler
                    for s in range(3):
                        for kb in range(KB):
                            z_mm(s, kb)
                    zwork = [(s, kb) for s in range(3, S)
                             for kb in range(KB)]
                    psums = {dc: upsum.tile([128, NPC], F32, tag="up",
                                            name=f"up_0_{dc}")
                             for dc in DCP[0]}
                    nfill = (len(zwork) + KB - 1) // KB
                    for kb in range(KB):
                        for dc in DCP[0]:
                            nc.tensor.matmul(
                                psums[dc][:],
                                z_sb[:, kb * SJ:kb * SJ + 128],
                                at_sb[:, kb * NP + dc * NPC:
                                      kb * NP + (dc + 1) * NPC],
                                start=(kb == 0), stop=(kb == KB - 1))
                        for _ in range(nfill):
                            if zwork:
                                z_mm(*zwork.pop(0))
                    for dc in DCP[0]:
                        st = stg0.tile([128, NPC], F16, tag="st")
                        nc.vector.tensor_copy(st[:], psums[dc][:])
                        stage(0, dc, st)
                    # blk0 pass2
                    psums2 = {dc: upsum.tile([128, NPC], F32, tag="up",
                                             name=f"up_0b_{dc}")
                              for dc in DCP[1]}
                    for kb in range(KB):
                        for dc in DCP[1]:
                            nc.tensor.matmul(
                                psums2[dc][:],
                                z_sb[:, kb * SJ:kb * SJ + 128],
                                at_sb[:, kb * NP + dc * NPC:
                                      kb * NP + (dc + 1) * NPC],
                                start=(kb == 0), stop=(kb == KB - 1))
                    for dc in DCP[1]:
                        st = stg0.tile([128, NPC], F16, tag="st")
                        nc.vector.tensor_copy(st[:], psums2[dc][:])
                        stage(0, dc, st)
                    trig(0)

            # ---- persistent pools (reuse xin's released space) ----
            ring_ctx = tc.tile_pool(name="ringp", bufs=1)
            ring_pool = ring_ctx.__enter__()
            stg_ctx = tc.tile_pool(name="stg", bufs=2)
            stg_pool = stg_ctx.__enter__()
            upool_ctx = tc.tile_pool(name="upool", bufs=2)
            state["upool"] = upool_ctx.__enter__()
            pp_ctx = tc.tile_pool(name="p3psum", bufs=2, space="PSUM")
            state["pp_pool"] = pp_ctx.__enter__()

            ring = ring_pool.tile([128, RING * NPC], F16, tag="ring")
            state["ring"] = ring
            nc.vector.memset(ring[:], 0.0)
            nc.sync.dma_start(ring[0:J, 0:NPC], h0T[:])  # h0 -> slot 0

            for b in range(PRE):
                prestage_u(b)

            # ================= U-phase blocks 1-6 ========================
            with nc.named_scope("ummphase"):
                for k in range(1, NMB):
                    row0 = k * 128
                    mrows = min(128, SJ - row0)
                    steps = list(WEAVE.get(k, []))
                    w1, wrest = steps[:12], steps[12:]
                    for pi, dcs in enumerate(DCP):
                        psums = {dc: upsum.tile([mrows, NPC], F32, tag="up",
                                                name=f"up_{k}_{dc}")
                                 for dc in dcs}
                        wq = w1 if pi == 0 else wrest
                        for kb in range(KB):
                            for dc in dcs:
                                nc.tensor.matmul(
                                    psums[dc][:],
                                    z_sb[:, kb * SJ + row0:
                                         kb * SJ + row0 + mrows],
                                    at_sb[:, kb * NP + dc * NPC:
                                          kb * NP + (dc + 1) * NPC],
                                    start=(kb == 0), stop=(kb == KB - 1))
                            if pi == 0:
                                if kb % 2 == 1 and wq:
                                    rnn_step(wq.pop(0))
                            elif wq:
                                rnn_step(wq.pop(0))
                        for dc in dcs:
                            st = stg_pool.tile([mrows, NPC], F16, tag="st")
                            nc.vector.tensor_copy(st[:], psums[dc][:])
                            stage(k, dc, st)
                        while wq:
                            rnn_step(wq.pop(0))
                    for r, blk in TRIG_BLK.items():
                        if blk == k:
                            trig(r)

            # ================= rnn tail ==================================
            with nc.named_scope("rnn"):
                for b in range(TAIL_B, B):
                    rnn_step(b)

            pp_ctx.__exit__(None, None, None)
            upool_ctx.__exit__(None, None, None)
            stg_ctx.__exit__(None, None, None)
            ring_ctx.__exit__(None, None, None)
            upsum_ctx.__exit__(None, None, None)

    nc.compile()
    _PROGRAM_CACHE["nc"] = nc
    return nc


def _host_prep(x_in, edge_index, edge_weight, W, b, W_ih, W_hh, b_ih, b_hh, h0):
    """Build per-core input maps (all numpy, no device work)."""
    edge_index = np.asarray(edge_index).astype(np.int64)
    # exact reference remap: rank among unique ids (size=N, fill=2**30)
    uniq = np.unique(edge_index)
    if uniq.size < N:
        uniq = np.concatenate([uniq, np.full(N - uniq.size, 2 ** 30, np.int64)])
    else:
        uniq = uniq[:N]
    ei = np.searchsorted(uniq, edge_index)
    src, dst = ei[0], ei[1]

    ew = np.asarray(edge_weight, np.float64)
    deg = np.zeros(N, np.float64)
    np.add.at(deg, dst, ew)
    deg += 1.0  # self loops (weight 1)
    dinv = np.where(deg > 0, 1.0 / np.sqrt(deg), 0.0)

    AT = np.zeros((NP, NP), np.float32)
    np.add.at(AT, (src, dst), (dinv[src] * ew * dinv[dst]).astype(np.float32))
    idx = np.arange(N)
    AT[idx, idx] += (dinv * dinv).astype(np.float32)
    AT16 = AT.astype(np.float16)

    W = np.asarray(W, np.float32)
    W_ih = np.asarray(W_ih, np.float32)
    W2 = (W.astype(np.float64) @ W_ih.T.astype(np.float64)).astype(np.float16)
    c0 = (np.asarray(b, np.float32) @ W_ih.T + np.asarray(b_ih, np.float32)
          + np.asarray(b_hh, np.float32)).astype(np.float32).reshape(J, 1)
    ws = np.zeros((128, J), np.float32)
    ws[0:J] = np.asarray(W_hh, np.float32).T
    ws[64:64 + J] = np.eye(J, dtype=np.float32)
    ws = ws.astype(np.float16)

    x_in = np.asarray(x_in, np.float32)
    h0 = np.asarray(h0, np.float32)
    h0p = np.zeros((NP, J), np.float16)
    h0p[:N] = h0.astype(np.float16)

    in_maps = []
    for c in range(NCORES):
        samples = [BOFF[r] + SR[r] * c + s4
                   for r in range(R) for s4 in range(SR[r])]
        xc = x_in[samples]                                # (S, N, F)
        xTc = np.ascontiguousarray(
            xc.transpose(0, 2, 1)).astype(np.float16)     # (S, F, N)
        h0Tc = np.ascontiguousarray(
            h0p[c * NPC:(c + 1) * NPC].T)                 # (J, NPC)
        in_maps.append({"xT": xTc, "at": AT16, "w2": W2, "ws": ws,
                        "c0": c0, "h0T": h0Tc})
    return in_maps


def _assemble(results):
    parts = []
    for c in range(NCORES):
        o = results[c]["out"]                 # (B, J, NPC) fp16
        parts.append(np.ascontiguousarray(o.transpose(0, 2, 1)))  # (B, NPC, J)
    full = np.concatenate(parts, axis=1)      # (B, NP, J)
    return full[:, :N, :].astype(np.float32)


def run_internal(inputs, trace=False, trace_cores=None):
    nc = _build_program()
    in_maps = _host_prep(**inputs)
    res = run_bass_kernel_spmd(nc, in_maps, list(range(NCORES)), trace=trace,
                               trace_cores=trace_cores)
    return _assemble(res.results), res


def kernel(**inputs) -> np.ndarray:
    out, _ = run_internal(inputs, trace=False)
    return out


# revision 8
# speedup vs baseline: 1.0496x; 1.0496x over previous
"""Trainium2 Bass kernel for GCN+RNN (nn_GCNN_RNN_32461362823865).

Strategy (v3):
  - Host: dense normalized adjacency A^T (fp16, 3072-padded) from the edge
    list (exact reference remap semantics), fold W2 = W @ W_ih.T and
    c0 = b @ W_ih.T + b_ih + b_hh, pre-transpose/cast x_in per core.
  - Startup window (DMA-bound ~90us): x on the scalar DMA queue, A^T on
    sync, issued interleaved so A is resident by window end; PE runs the
    (LDW-paced, HAM-cold) z = x@W2 stream which just fits the window.
    A dummy 1KB AllToAll absorbs the collective warmup + barrier.
  - U-phase: U^T = z^T x A^T in 7 M-blocks x (6+2 dc passes) x 24 kb,
    fp16 384-wide MMs at the 162ns roofline. Staging casts psum->fp16
    into 6 AllToAll rounds of (2,3,5,2,3,1) samples, each triggered at
    the M-block where its rows complete.
  - RNN rides a ring buffer [128 part, 12 slots x 384]: rows 0:50 = h
    (written directly by ScalarE tanh), rows 64:114 = U, DMA'd straight
    from the AllToAll output into the slot (gpsimd queue; emission gated
    behind each round's trigger to keep that queue deadlock-free).
    One 128-contraction MM per step computes W_hh^T h + I*U; woven steps
    are 1 MM + 1 ACT (N=384); tail steps split into column halves with
    two psum pools (separate banks) to decouple MM/ACT latency.
    Output groups DMA straight out of the ring.

  Sample->core map: core c, round r holds global samples
  boff[r] + SR[r]*c + s4, boff = [0,16,40,80,96,120], SR = (2,3,5,2,3,1).
"""
import numpy as np

import concourse.bacc as bacc
import concourse.mybir as mybir
from concourse import tile
from concourse.bass_utils import run_bass_kernel_spmd

# ---- problem constants (hardcoded per contract) ----
N = 3070          # nodes
NP = 3072         # padded nodes (24 * 128, 8 * 384)
F = 128           # input features
J = 50            # folded feature dim (= RNN hidden)
B = 128           # batch (RNN sequence length)
NCORES = 8
S = B // NCORES   # samples per core = 16
NPC = NP // NCORES  # nodes per core = 384
KB = NP // 128    # 24 contraction blocks
SJ = S * J        # 800 U^T rows per core
NMB = 7           # M-blocks of 128 rows (last = 32)

SR = (2, 3, 5, 2, 3, 1)          # samples per round per core
R = len(SR)
BOFF = [0, 16, 40, 80, 96, 120]  # global step offset per round
ROFF = [0, 100, 250, 500, 600, 750, 800]   # U^T row offset per round
RJ = [SR[r] * J for r in range(R)]
TRIG_BLK = {0: 0, 1: 1, 2: 3, 3: 4, 4: 5, 5: 6}  # round -> trigger M-block
DCP = [list(range(6)), [6, 7]]   # dest-core passes (6+2 psum banks)

RING = 12         # rnn ring slots
TAIL_B = 100      # steps >= TAIL_B use the split (2 MM + 2 ACT) form
PRE = 8           # U prestage distance (steps ahead)

# weave schedule: M-block -> rnn steps woven into it (first 12 into pass1
# at 1-per-2kb, rest into pass2 at 1-per-kb)
WEAVE = {3: list(range(0, 24)), 4: list(range(24, 40)),
         5: list(range(40, 64)), 6: list(range(64, 100))}

F16 = mybir.dt.float16
F32 = mybir.dt.float32
TANH = mybir.ActivationFunctionType.Tanh

_PROGRAM_CACHE = {}


def _step_rc(b):
    """global step -> (round, src core, sample-within-block)."""
    r = 0
    while r + 1 < R and b >= BOFF[r + 1]:
        r += 1
    q = b - BOFF[r]
    return r, q // SR[r], q % SR[r]


# writeback groups aligned so ring slots never wrap mid-group:
# out[b] lives in slot (b+1) % RING rows 0:50.
WB_GROUPS = [(0, 3)] + [(3 + 4 * i, 4) for i in range(31)] + [(127, 1)]
WB_AFTER = {b0 + g - 1: (b0, g) for b0, g in WB_GROUPS}


def _build_program():
    if "nc" in _PROGRAM_CACHE:
        return _PROGRAM_CACHE["nc"]
    nc = bacc.Bacc("TRN2", target_bir_lowering=False, debug=False,
                   num_devices=NCORES)

    xT = nc.dram_tensor("xT", [S, F, N], F16, kind="ExternalInput")
    at = nc.dram_tensor("at", [NP, NP], F16, kind="ExternalInput")
    w2 = nc.dram_tensor("w2", [F, J], F16, kind="ExternalInput")
    ws = nc.dram_tensor("ws", [128, J], F16, kind="ExternalInput")
    c0 = nc.dram_tensor("c0", [J, 1], F32, kind="ExternalInput")
    h0T = nc.dram_tensor("h0T", [J, NPC], F16, kind="ExternalInput")
    out = nc.dram_tensor("out", [B, J, NPC], F16, kind="ExternalOutput")

    with tile.TileContext(nc) as tc:
        with (
            tc.tile_pool(name="consts", bufs=1) as consts,
            tc.tile_pool(name="persist", bufs=1) as persist,
            tc.tile_pool(name="dram", bufs=1, space="DRAM") as dram,
        ):
            w2_sb = consts.tile([F, J], F16, tag="w2_sb")
            ws_sb = consts.tile([128, J], F16, tag="ws_sb")
            c0_sb = consts.tile([J, 1], F32, tag="c0_sb")
            nc.sync.dma_start(w2_sb[:], w2[:])
            nc.sync.dma_start(ws_sb[:], ws[:])
            nc.sync.dma_start(c0_sb[:], c0[:])

            at_sb = persist.tile([128, KB * NP], F16, tag="at_sb")
            z_sb = persist.tile([128, KB * SJ], F16, tag="z_sb")

            a2a_in = [dram.tile([NCORES * RJ[r], NPC], F16, name=f"a2ai_{r}")
                      for r in range(R)]
            a2a_out = [dram.tile([NCORES * RJ[r], NPC], F16, name=f"a2ao_{r}")
                       for r in range(R)]
            warm_in = dram.tile([NCORES, 64], F16, name="warm_in")
            warm_out = dram.tile([NCORES, 64], F16, name="warm_out")

            upsum_ctx = tc.tile_pool(name="upsum", bufs=6, space="PSUM")
            upsum = upsum_ctx.__enter__()

            # dummy collective: absorbs barrier + mesh warmup in the
            # DMA-bound startup window
            nc.gpsimd.collective_compute(
                "AllToAll", mybir.AluOpType.bypass,
                replica_groups=[list(range(NCORES))],
                ins=[warm_in.opt()], outs=[warm_out.opt()])

            state = {}
            trig_done = [False] * R
            pending_pre = []

            def _do_prestage(b):
                r, c, s4 = _step_rc(b)
                slot = b % RING
                row = c * RJ[r] + s4 * J
                nc.gpsimd.dma_start(
                    state["ring"][64:64 + J, slot * NPC:(slot + 1) * NPC],
                    a2a_out[r][row:row + J, :])

            def prestage_u(b):
                if trig_done[_step_rc(b)[0]]:
                    _do_prestage(b)
                else:
                    pending_pre.append(b)

            def trig(r):
                nc.gpsimd.collective_compute(
                    "AllToAll", mybir.AluOpType.bypass,
                    replica_groups=[list(range(NCORES))],
                    ins=[a2a_in[r].opt()],
                    outs=[a2a_out[r].opt()])
                trig_done[r] = True
                ready = [b for b in pending_pre if trig_done[_step_rc(b)[0]]]
                for b in ready:
                    pending_pre.remove(b)
                    _do_prestage(b)

            def rnn_step(b):
                slot = b % RING
                nslot = (b + 1) % RING
                if b + PRE < B:
                    prestage_u(b + PRE)
                ring = state["ring"]
                rhs = ring[:, slot * NPC:(slot + 1) * NPC]
                dst = ring[0:J, nslot * NPC:(nslot + 1) * NPC]
                if b < TAIL_B:
                    pp = state["pp"].tile([J, NPC], F32, tag="pp",
                                          name=f"pp_{b}")
                    nc.tensor.matmul(pp[:], ws_sb[:], rhs,
                                     start=True, stop=True)
                    nc.scalar.activation(dst, pp[:], TANH,
                                         bias=c0_sb[:, 0:1])
                else:
                    # split form: halves alternate the pp pool's two slots
                    # (distinct banks), so ACT(h0) runs beside MM(h1) and
                    # slot reuse coincides with the h-data dependency.
                    H = NPC // 2
                    for half in range(2):
                        pph = state["pp"].tile([J, NPC], F32, tag="pp",
                                               name=f"pp_{b}_{half}")
                        nc.tensor.matmul(pph[:, 0:H], ws_sb[:],
                                         rhs[:, half * H:(half + 1) * H],
                                         start=True, stop=True)
                        nc.scalar.activation(
                            dst[:, half * H:(half + 1) * H], pph[:, 0:H],
                            TANH, bias=c0_sb[:, 0:1])
                if b in WB_AFTER:
                    b0, g = WB_AFTER[b]
                    s0 = (b0 + 1) % RING
                    nc.sync.dma_start(
                        out[b0:b0 + g].rearrange("g j n -> j g n"),
                        ring[0:J, s0 * NPC:(s0 + g) * NPC].rearrange(
                            "j (g n) -> j g n", g=g))

            def stage(k, dc, st):
                """staging DMAs for M-block k, dest core dc (rows split
                at round boundaries)."""
                row0 = k * 128
                mrows = min(128, SJ - row0)
                for r in range(R):
                    lo = max(row0, ROFF[r])
                    hi = min(row0 + mrows, ROFF[r + 1])
                    if lo >= hi:
                        continue
                    nc.sync.dma_start(
                        a2a_in[r][dc * RJ[r] + lo - ROFF[r]:
                                  dc * RJ[r] + hi - ROFF[r], :],
                        st[lo - row0:hi - row0, :])

            # ================= startup: x/A DMA + z =====================
            with nc.named_scope("startup"):
                with (
                    tc.tile_pool(name="xin", bufs=3) as xin,
                    tc.tile_pool(name="zpsum", bufs=2, space="PSUM") as zpsum,
                ):
                    xbigs = {}

                    def load_x(s):
                        xb = xin.tile([F, NP], F16, tag="xbig",
                                      name=f"xbig_{s}")
                        xbigs[s] = xb
                        nc.scalar.dma_start(xb[:, 0:N], xT[s])
                        nc.vector.memset(xb[:, N:NP], 0.0)

                    # DMA order: x s0-2 (scalar q), A blocks (sync q)
                    # interleaved 2:1 with remaining x
                    for s in range(3):
                        load_x(s)
                    xq = list(range(3, S))
                    for kb in range(KB):
                        nc.sync.dma_start(
                            at_sb[:, kb * NP:(kb + 1) * NP],
                            at[kb * 128:(kb + 1) * 128, :])
                        if kb % 2 == 0 and xq:
                            load_x(xq.pop(0))
                    while xq:
                        load_x(xq.pop(0))

                    for s in range(S):
                        for kb in range(KB):
                            zp = zpsum.tile([128, J], F32, tag="zp",
                                            name=f"zp_{s}_{kb}")
                            nc.tensor.matmul(
                                zp[:], xbigs[s][:, kb * 128:(kb + 1) * 128],
                                w2_sb[:], start=True, stop=True)
                            nc.vector.tensor_copy(
                                z_sb[:, kb * SJ + s * J:
                                     kb * SJ + (s + 1) * J], zp[:])

            # ---- persistent pools (reuse xin's released space) ----
            ring_ctx = tc.tile_pool(name="ringp", bufs=1)
            ring_pool = ring_ctx.__enter__()
            stg_ctx = tc.tile_pool(name="stg", bufs=2)
            stg_pool = stg_ctx.__enter__()
            pp_ctx = tc.tile_pool(name="p3psum", bufs=2, space="PSUM")
            state["pp"] = pp_ctx.__enter__()

            ring = ring_pool.tile([128, RING * NPC], F16, tag="ring")
            state["ring"] = ring
            nc.vector.memset(ring[:], 0.0)
            nc.sync.dma_start(ring[0:J, 0:NPC], h0T[:])  # h0 -> slot 0

            for b in range(PRE):
                prestage_u(b)

            # ================= U-phase blocks 0-6 ========================
            with nc.named_scope("ummphase"):
                for k in range(NMB):
                    row0 = k * 128
                    mrows = min(128, SJ - row0)
                    steps = list(WEAVE.get(k, []))
                    w1, wrest = steps[:12], steps[12:]
                    for pi, dcs in enumerate(DCP):
                        psums = {dc: upsum.tile([mrows, NPC], F32, tag="up",
                                                name=f"up_{k}_{dc}")
                                 for dc in dcs}
                        wq = w1 if pi == 0 else wrest
                        for kb in range(KB):
                            for dc in dcs:
                                nc.tensor.matmul(
                                    psums[dc][:],
                                    z_sb[:, kb * SJ + row0:
                                         kb * SJ + row0 + mrows],
                                    at_sb[:, kb * NP + dc * NPC:
                                          kb * NP + (dc + 1) * NPC],
                                    start=(kb == 0), stop=(kb == KB - 1))
                            if pi == 0:
                                if kb % 2 == 1 and wq:
                                    rnn_step(wq.pop(0))
                            elif wq:
                                rnn_step(wq.pop(0))
                        for dc in dcs:
                            st = stg_pool.tile([mrows, NPC], F16, tag="st")
                            nc.vector.tensor_copy(st[:], psums[dc][:])
                            stage(k, dc, st)
                        while wq:
                            rnn_step(wq.pop(0))
                    for r, blk in TRIG_BLK.items():
                        if blk == k:
                            trig(r)

            # ================= rnn tail ==================================
            with nc.named_scope("rnn"):
                for b in range(TAIL_B, B):
                    rnn_step(b)

            pp_ctx.__exit__(None, None, None)
            stg_ctx.__exit__(None, None, None)
            ring_ctx.__exit__(None, None, None)
            upsum_ctx.__exit__(None, None, None)

    nc.compile()
    _PROGRAM_CACHE["nc"] = nc
    return nc


def _host_prep(x_in, edge_index, edge_weight, W, b, W_ih, W_hh, b_ih, b_hh, h0):
    """Build per-core input maps (all numpy, no device work)."""
    edge_index = np.asarray(edge_index).astype(np.int64)
    # exact reference remap: rank among unique ids (size=N, fill=2**30)
    uniq = np.unique(edge_index)
    if uniq.size < N:
        uniq = np.concatenate([uniq, np.full(N - uniq.size, 2 ** 30, np.int64)])
    else:
        uniq = uniq[:N]
    ei = np.searchsorted(uniq, edge_index)
    src, dst = ei[0], ei[1]

    ew = np.asarray(edge_weight, np.float64)
    deg = np.zeros(N, np.float64)
    np.add.at(deg, dst, ew)
    deg += 1.0  # self loops (weight 1)
    dinv = np.where(deg > 0, 1.0 / np.sqrt(deg), 0.0)

    AT = np.zeros((NP, NP), np.float32)
    np.add.at(AT, (src, dst), (dinv[src] * ew * dinv[dst]).astype(np.float32))
    idx = np.arange(N)
    AT[idx, idx] += (dinv * dinv).astype(np.float32)
    AT16 = AT.astype(np.float16)

    W = np.asarray(W, np.float32)
    W_ih = np.asarray(W_ih, np.float32)
    W2 = (W.astype(np.float64) @ W_ih.T.astype(np.float64)).astype(np.float16)
    c0 = (np.asarray(b, np.float32) @ W_ih.T + np.asarray(b_ih, np.float32)
          + np.asarray(b_hh, np.float32)).astype(np.float32).reshape(J, 1)
    ws = np.zeros((128, J), np.float32)
    ws[0:J] = np.asarray(W_hh, np.float32).T
    ws[64:64 + J] = np.eye(J, dtype=np.float32)
    ws = ws.astype(np.float16)

    x_in = np.asarray(x_in, np.float32)
    h0 = np.asarray(h0, np.float32)
    h0p = np.zeros((NP, J), np.float16)
    h0p[:N] = h0.astype(np.float16)

    in_maps = []
    for c in range(NCORES):
        samples = [BOFF[r] + SR[r] * c + s4
                   for r in range(R) for s4 in range(SR[r])]
        xc = x_in[samples]                                # (S, N, F)
        xTc = np.ascontiguousarray(
            xc.transpose(0, 2, 1)).astype(np.float16)     # (S, F, N)
        h0Tc = np.ascontiguousarray(
            h0p[c * NPC:(c + 1) * NPC].T)                 # (J, NPC)
        in_maps.append({"xT": xTc, "at": AT16, "w2": W2, "ws": ws,
                        "c0": c0, "h0T": h0Tc})
    return in_maps


def _assemble(results):
    parts = []
    for c in range(NCORES):
        o = results[c]["out"]                 # (B, J, NPC) fp16
        parts.append(np.ascontiguousarray(o.transpose(0, 2, 1)))  # (B, NPC, J)
    full = np.concatenate(parts, axis=1)      # (B, NP, J)
    return full[:, :N, :].astype(np.float32)


def run_internal(inputs, trace=False, trace_cores=None):
    nc = _build_program()
    in_maps = _host_prep(**inputs)
    res = run_bass_kernel_spmd(nc, in_maps, list(range(NCORES)), trace=trace,
                               trace_cores=trace_cores)
    return _assemble(res.results), res


def kernel(**inputs) -> np.ndarray:
    out, _ = run_internal(inputs, trace=False)
    return out


# revision 15
# speedup vs baseline: 1.1195x; 1.0666x over previous
"""Trainium2 Bass kernel for GCN+RNN (nn_GCNN_RNN_32461362823865).

Strategy (v3):
  - Host: dense normalized adjacency A^T (fp16, 3072-padded) from the edge
    list (exact reference remap semantics), fold W2 = W @ W_ih.T and
    c0 = b @ W_ih.T + b_ih + b_hh, pre-transpose/cast x_in per core.
  - Startup window (DMA-bound ~90us): x on the scalar DMA queue, A^T on
    sync, issued interleaved so A is resident by window end; PE runs the
    (LDW-paced, HAM-cold) z = x@W2 stream which just fits the window.
    A dummy 1KB AllToAll absorbs the collective warmup + barrier.
  - U-phase: U^T = z^T x A^T in 7 M-blocks x (6+2 dc passes) x 24 kb,
    fp16 384-wide MMs at the 162ns roofline. Staging casts psum->fp16
    into 6 AllToAll rounds of (2,3,5,2,3,1) samples, each triggered at
    the M-block where its rows complete.
  - RNN rides a ring buffer [128 part, 12 slots x 384]: rows 0:50 = h
    (written directly by ScalarE tanh), rows 64:114 = U, DMA'd straight
    from the AllToAll output into the slot (gpsimd queue; emission gated
    behind each round's trigger to keep that queue deadlock-free).
    One 128-contraction MM per step computes W_hh^T h + I*U; woven steps
    are 1 MM + 1 ACT (N=384); tail steps split into column halves with
    two psum pools (separate banks) to decouple MM/ACT latency.
    Output groups DMA straight out of the ring.

  Sample->core map: core c, round r holds global samples
  boff[r] + SR[r]*c + s4, boff = [0,16,40,80,96,120], SR = (2,3,5,2,3,1).
"""
import numpy as np

import concourse.bacc as bacc
import concourse.mybir as mybir
from concourse import tile
from concourse.bass_utils import run_bass_kernel_spmd

# ---- problem constants (hardcoded per contract) ----
N = 3070          # nodes
NP = 3072         # padded nodes (24 * 128, 8 * 384)
F = 128           # input features
J = 50            # folded feature dim (= RNN hidden)
B = 128           # batch (RNN sequence length)
NCORES = 8
S = B // NCORES   # samples per core = 16
NPC = NP // NCORES  # nodes per core = 384
KB = NP // 128    # 24 contraction blocks
SJ = S * J        # 800 U^T rows per core
NMB = 7           # M-blocks of 128 rows (last = 32)

SR = (2, 3, 5, 2, 3, 1)          # samples per round per core
R = len(SR)
BOFF = [0, 16, 40, 80, 96, 120]  # global step offset per round
ROFF = [0, 100, 250, 500, 600, 750, 800]   # U^T row offset per round
RJ = [SR[r] * J for r in range(R)]
TRIG_BLK = {0: 0, 1: 1, 2: 3, 3: 4, 4: 5, 5: 6}  # round -> trigger M-block
DCP = [list(range(6)), [6, 7]]   # dest-core passes (6+2 psum banks)

RING = 12         # rnn ring slots
TAIL_B = 100      # steps >= TAIL_B use the split (2 MM + 2 ACT) form
PRE = 8           # U prestage distance (steps ahead)

# weave schedule: M-block -> rnn steps woven into it (first 12 into pass1
# at 1-per-2kb, rest into pass2 at 1-per-kb)
WEAVE = {3: list(range(0, 24)), 4: list(range(24, 40)),
         5: list(range(40, 64)), 6: list(range(64, 100))}

F16 = mybir.dt.float16
F32 = mybir.dt.float32
TANH = mybir.ActivationFunctionType.Tanh

_PROGRAM_CACHE = {}


def _step_rc(b):
    """global step -> (round, src core, sample-within-block)."""
    r = 0
    while r + 1 < R and b >= BOFF[r + 1]:
        r += 1
    q = b - BOFF[r]
    return r, q // SR[r], q % SR[r]


# writeback groups aligned so ring slots never wrap mid-group:
# out[b] lives in slot (b+1) % RING rows 0:50.
WB_GROUPS = [(0, 3)] + [(3 + 4 * i, 4) for i in range(31)] + [(127, 1)]
WB_AFTER = {b0 + g - 1: (b0, g) for b0, g in WB_GROUPS}


def _build_program():
    if "nc" in _PROGRAM_CACHE:
        return _PROGRAM_CACHE["nc"]
    nc = bacc.Bacc("TRN2", target_bir_lowering=False, debug=False,
                   num_devices=NCORES)

    xT = nc.dram_tensor("xT", [S, F, N], F16, kind="ExternalInput")
    at = nc.dram_tensor("at", [NP, NP], F16, kind="ExternalInput")
    w2 = nc.dram_tensor("w2", [F, J], F16, kind="ExternalInput")
    ws = nc.dram_tensor("ws", [128, J], F16, kind="ExternalInput")
    c0 = nc.dram_tensor("c0", [J, 1], F32, kind="ExternalInput")
    h0T = nc.dram_tensor("h0T", [J, NPC], F16, kind="ExternalInput")
    out = nc.dram_tensor("out", [B, J, NPC], F16, kind="ExternalOutput")

    with tile.TileContext(nc) as tc:
        with (
            tc.tile_pool(name="consts", bufs=1) as consts,
            tc.tile_pool(name="persist", bufs=1) as persist,
            tc.tile_pool(name="dram", bufs=1, space="DRAM") as dram,
        ):
            w2_sb = consts.tile([F, J], F16, tag="w2_sb")
            ws_sb = consts.tile([128, J], F16, tag="ws_sb")
            c0_sb = consts.tile([J, 1], F32, tag="c0_sb")
            nc.sync.dma_start(w2_sb[:], w2[:])
            nc.sync.dma_start(ws_sb[:], ws[:])
            nc.sync.dma_start(c0_sb[:], c0[:])

            at_sb = persist.tile([128, KB * NP], F16, tag="at_sb")
            z_sb = persist.tile([128, KB * SJ], F16, tag="z_sb")

            a2a_in = [dram.tile([NCORES * RJ[r], NPC], F16, name=f"a2ai_{r}")
                      for r in range(R)]
            a2a_out = [dram.tile([NCORES * RJ[r], NPC], F16, name=f"a2ao_{r}")
                       for r in range(R)]
            warm_in = dram.tile([NCORES, 8192], F16, name="warm_in")
            warm_out = dram.tile([NCORES, 8192], F16, name="warm_out")

            # dummy collective: absorbs barrier + mesh warmup in the
            # DMA-bound startup window
            nc.gpsimd.collective_compute(
                "AllToAll", mybir.AluOpType.bypass,
                replica_groups=[list(range(NCORES))],
                ins=[warm_in.opt()], outs=[warm_out.opt()])

            state = {}
            trig_done = [False] * R
            pending_pre = []

            def _do_prestage(b):
                r, c, s4 = _step_rc(b)
                slot = b % RING
                row = c * RJ[r] + s4 * J
                nc.gpsimd.dma_start(
                    state["ring"][64:64 + J, slot * NPC:(slot + 1) * NPC],
                    a2a_out[r][row:row + J, :])

            def prestage_u(b):
                if trig_done[_step_rc(b)[0]]:
                    _do_prestage(b)
                else:
                    pending_pre.append(b)

            def trig(r):
                nc.gpsimd.collective_compute(
                    "AllToAll", mybir.AluOpType.bypass,
                    replica_groups=[list(range(NCORES))],
                    ins=[a2a_in[r].opt()],
                    outs=[a2a_out[r].opt()])
                trig_done[r] = True
                ready = [b for b in pending_pre if trig_done[_step_rc(b)[0]]]
                for b in ready:
                    pending_pre.remove(b)
                    _do_prestage(b)

            def rnn_step(b):
                slot = b % RING
                nslot = (b + 1) % RING
                if b + PRE < B:
                    prestage_u(b + PRE)
                ring = state["ring"]
                rhs = ring[:, slot * NPC:(slot + 1) * NPC]
                dst = ring[0:J, nslot * NPC:(nslot + 1) * NPC]
                if b < TAIL_B:
                    pp = state["pp"].tile([J, NPC], F32, tag="pp",
                                          name=f"pp_{b}")
                    nc.tensor.matmul(pp[:], ws_sb[:], rhs,
                                     start=True, stop=True)
                    nc.scalar.activation(dst, pp[:], TANH,
                                         bias=c0_sb[:, 0:1])
                else:
                    # split form: halves alternate the pp pool's two slots
                    # (distinct banks), so ACT(h0) runs beside MM(h1) and
                    # slot reuse coincides with the h-data dependency.
                    H = NPC // 2
                    for half in range(2):
                        pph = state["pp"].tile([J, NPC], F32, tag="pp",
                                               name=f"pp_{b}_{half}")
                        nc.tensor.matmul(pph[:, 0:H], ws_sb[:],
                                         rhs[:, half * H:(half + 1) * H],
                                         start=True, stop=True)
                        nc.scalar.activation(
                            dst[:, half * H:(half + 1) * H], pph[:, 0:H],
                            TANH, bias=c0_sb[:, 0:1])
                if b in WB_AFTER:
                    b0, g = WB_AFTER[b]
                    s0 = (b0 + 1) % RING
                    nc.sync.dma_start(
                        out[b0:b0 + g].rearrange("g j n -> j g n"),
                        ring[0:J, s0 * NPC:(s0 + g) * NPC].rearrange(
                            "j (g n) -> j g n", g=g))

            def stage(k, dcs, st):
                """staging DMAs for M-block k covering all dest cores in
                `dcs` from the combined cast tile st [mrows, len(dcs)*NPC]
                (rows split at round boundaries; one strided DMA per
                round chunk)."""
                row0 = k * 128
                mrows = min(128, SJ - row0)
                nd = len(dcs)
                dc0 = dcs[0]          # dcs are consecutive
                eng = nc.sync if len(dcs) > 2 else nc.scalar
                for r in range(R):
                    lo = max(row0, ROFF[r])
                    hi = min(row0 + mrows, ROFF[r + 1])
                    if lo >= hi:
                        continue
                    eng.dma_start(
                        a2a_in[r].rearrange("(dc rw) n -> rw dc n",
                                            dc=NCORES)[
                            lo - ROFF[r]:hi - ROFF[r], dc0:dc0 + nd, :],
                        st[lo - row0:hi - row0, :].rearrange(
                            "rw (dc n) -> rw dc n", dc=nd))

            # ================= startup: x/A DMA + z =====================
            with nc.named_scope("startup"):
                with (
                    tc.tile_pool(name="xin", bufs=3) as xin,
                    tc.tile_pool(name="zpsum", bufs=6, space="PSUM") as zpsum,
                ):
                    xbigs = {}

                    def load_x(s):
                        xb = xin.tile([F, NP], F16, tag="xbig",
                                      name=f"xbig_{s}")
                        xbigs[s] = xb
                        nc.scalar.dma_start(xb[:, 0:N], xT[s])
                        nc.vector.memset(xb[:, N:NP], 0.0)

                    # DMA order: x s0-2 (scalar q), A blocks (sync q)
                    # interleaved 3:1 with remaining x
                    for s in range(3):
                        load_x(s)
                    xq = list(range(3, S))
                    for kb in range(KB):
                        nc.sync.dma_start(
                            at_sb[:, kb * NP:(kb + 1) * NP],
                            at[kb * 128:(kb + 1) * 128, :])
                        if kb % 3 == 2 and xq:
                            load_x(xq.pop(0))
                    while xq:
                        load_x(xq.pop(0))

                    for s in range(S):
                        for kb in range(KB):
                            zp = zpsum.tile([128, J], F32, tag="zp",
                                            name=f"zp_{s}_{kb}")
                            nc.tensor.matmul(
                                zp[:], xbigs[s][:, kb * 128:(kb + 1) * 128],
                                w2_sb[:], start=True, stop=True)
                            nc.vector.tensor_copy(
                                z_sb[:, kb * SJ + s * J:
                                     kb * SJ + (s + 1) * J], zp[:])

            # ---- persistent pools (reuse xin's released space) ----
            upsum_ctx = tc.tile_pool(name="upsum", bufs=6, space="PSUM")
            upsum = upsum_ctx.__enter__()
            ring_ctx = tc.tile_pool(name="ringp", bufs=1)
            ring_pool = ring_ctx.__enter__()
            stg_ctx = tc.tile_pool(name="stg", bufs=2)
            stg_pool = stg_ctx.__enter__()
            pp_ctx = tc.tile_pool(name="p3psum", bufs=2, space="PSUM")
            state["pp"] = pp_ctx.__enter__()

            ring = ring_pool.tile([128, RING * NPC], F16, tag="ring")
            state["ring"] = ring
            nc.vector.memset(ring[:], 0.0)
            nc.sync.dma_start(ring[0:J, 0:NPC], h0T[:])  # h0 -> slot 0

            for b in range(PRE):
                prestage_u(b)

            # ================= U-phase blocks 0-6 ========================
            with nc.named_scope("ummphase"):
                for k in range(NMB):
                    row0 = k * 128
                    mrows = min(128, SJ - row0)
                    steps = list(WEAVE.get(k, []))
                    w1, wrest = steps[:12], steps[12:]
                    for pi, dcs in enumerate(DCP):
                        psums = {dc: upsum.tile([mrows, NPC], F32, tag="up",
                                                name=f"up_{k}_{dc}")
                                 for dc in dcs}
                        wq = w1 if pi == 0 else wrest
                        for kb in range(KB):
                            for dc in dcs:
                                nc.tensor.matmul(
                                    psums[dc][:],
                                    z_sb[:, kb * SJ + row0:
                                         kb * SJ + row0 + mrows],
                                    at_sb[:, kb * NP + dc * NPC:
                                          kb * NP + (dc + 1) * NPC],
                                    start=(kb == 0), stop=(kb == KB - 1))
                            if pi == 0:
                                if kb % 2 == 1 and wq:
                                    rnn_step(wq.pop(0))
                            elif wq:
                                rnn_step(wq.pop(0))
                        st = stg_pool.tile([128, 6 * NPC], F16, tag="st",
                                           name=f"st_{k}_{pi}")
                        for di, dc in enumerate(dcs):
                            nc.vector.tensor_copy(
                                st[0:mrows, di * NPC:(di + 1) * NPC],
                                psums[dc][:])
                        stage(k, dcs, st[0:mrows, 0:len(dcs) * NPC])
                        while wq:
                            rnn_step(wq.pop(0))
                    for r, blk in TRIG_BLK.items():
                        if blk == k:
                            trig(r)

            # ================= rnn tail ==================================
            with nc.named_scope("rnn"):
                for b in range(TAIL_B, B):
                    rnn_step(b)

            pp_ctx.__exit__(None, None, None)
            stg_ctx.__exit__(None, None, None)
            ring_ctx.__exit__(None, None, None)
            upsum_ctx.__exit__(None, None, None)

    nc.compile()
    _PROGRAM_CACHE["nc"] = nc
    return nc


def _host_prep(x_in, edge_index, edge_weight, W, b, W_ih, W_hh, b_ih, b_hh, h0):
    """Build per-core input maps (all numpy, no device work)."""
    edge_index = np.asarray(edge_index).astype(np.int64)
    # exact reference remap: rank among unique ids (size=N, fill=2**30)
    uniq = np.unique(edge_index)
    if uniq.size < N:
        uniq = np.concatenate([uniq, np.full(N - uniq.size, 2 ** 30, np.int64)])
    else:
        uniq = uniq[:N]
    ei = np.searchsorted(uniq, edge_index)
    src, dst = ei[0], ei[1]

    ew = np.asarray(edge_weight, np.float64)
    deg = np.zeros(N, np.float64)
    np.add.at(deg, dst, ew)
    deg += 1.0  # self loops (weight 1)
    dinv = np.where(deg > 0, 1.0 / np.sqrt(deg), 0.0)

    AT = np.zeros((NP, NP), np.float32)
    np.add.at(AT, (src, dst), (dinv[src] * ew * dinv[dst]).astype(np.float32))
    idx = np.arange(N)
    AT[idx, idx] += (dinv * dinv).astype(np.float32)
    AT16 = AT.astype(np.float16)

    W = np.asarray(W, np.float32)
    W_ih = np.asarray(W_ih, np.float32)
    W2 = (W.astype(np.float64) @ W_ih.T.astype(np.float64)).astype(np.float16)
    c0 = (np.asarray(b, np.float32) @ W_ih.T + np.asarray(b_ih, np.float32)
          + np.asarray(b_hh, np.float32)).astype(np.float32).reshape(J, 1)
    ws = np.zeros((128, J), np.float32)
    ws[0:J] = np.asarray(W_hh, np.float32).T
    ws[64:64 + J] = np.eye(J, dtype=np.float32)
    ws = ws.astype(np.float16)

    x_in = np.asarray(x_in, np.float32)
    h0 = np.asarray(h0, np.float32)
    h0p = np.zeros((NP, J), np.float16)
    h0p[:N] = h0.astype(np.float16)

    in_maps = []
    for c in range(NCORES):
        samples = [BOFF[r] + SR[r] * c + s4
                   for r in range(R) for s4 in range(SR[r])]
        xc = x_in[samples]                                # (S, N, F)
        xTc = np.ascontiguousarray(
            xc.transpose(0, 2, 1)).astype(np.float16)     # (S, F, N)
        h0Tc = np.ascontiguousarray(
            h0p[c * NPC:(c + 1) * NPC].T)                 # (J, NPC)
        in_maps.append({"xT": xTc, "at": AT16, "w2": W2, "ws": ws,
                        "c0": c0, "h0T": h0Tc})
    return in_maps


def _assemble(results):
    parts = []
    for c in range(NCORES):
        o = results[c]["out"]                 # (B, J, NPC) fp16
        parts.append(np.ascontiguousarray(o.transpose(0, 2, 1)))  # (B, NPC, J)
    full = np.concatenate(parts, axis=1)      # (B, NP, J)
    return full[:, :N, :].astype(np.float32)


def run_internal(inputs, trace=False, trace_cores=None):
    nc = _build_program()
    in_maps = _host_prep(**inputs)
    res = run_bass_kernel_spmd(nc, in_maps, list(range(NCORES)), trace=trace,
                               trace_cores=trace_cores)
    return _assemble(res.results), res


def kernel(**inputs) -> np.ndarray:
    out, _ = run_internal(inputs, trace=False)
    return out
